# Initial kernel scaffold
#
"""Trainium2 Bass kernel for nn_LocalRNN (local GRU, chunked scan).

Problem: B=32, S=2048, I=H=256, ksize=16. Each ksize-chunk runs a GRU from
h0=0, so the 32*128=4096 chunks are independent length-16 GRU chains.

Sharding: data-parallel over chunks — core c gets batch rows [4c:4c+4],
i.e. 512 chains. Weights replicated.

Per-core kernel layout ("transposed"): gate/hidden dim on partitions, chain
(seq) index on the free dim. Per step t and seq-group g (2 groups x 256 seqs):

  gates[3H, seqs] = W_ih @ x_t^T + W_hh @ h_{t-1}^T     (PSUM accumulation)
  r = sigmoid(psum_r + (b_ih+b_hh)_r)                    (ScalarE, bias port)
  z = sigmoid(psum_z + (b_ih+b_hh)_z)
  n = tanh((psum_in + b_ih_n) + r*(psum_hn + b_hh_n))    (fused DVE stt ops)
  h = n + z*(h_prev - n)

The x-side and h-side matmuls for r/z accumulate into the same PSUM bank so
no explicit adds are needed; n keeps separate x/h banks because r multiplies
only the h side. PSUM budget: 4 banks per group x 2 groups = all 8 banks,
ping-ponged so one group's matmuls overlap the other group's elementwise.

Matmul operands and SBUF elementwise tensors are fp16 (PE fast-weight-load +
DVE 2x mode, ~8x finer mantissa than bf16; values are O(1) so fp16 range is
safe); PSUM accumulation is fp32. Host pre-transposes x / weights into
DMA-friendly contiguous blocks and inverts the output layout at the end.
"""

import sys

for _p in ("/opt/trn_rl_repo", "/root/.axon_site"):
    if _p not in sys.path:
        sys.path.insert(0, _p)

import ml_dtypes
import numpy as np

import concourse.bass as bass  # noqa: F401
import concourse.tile as tile
from concourse import bacc, mybir
from concourse.bass_utils import run_bass_kernel_spmd

# Problem constants (hardcoded per harness contract).
B, S, I, H = 32, 2048, 256, 256
KSIZE = 16
NCORES = 8
ROWS_PER_CORE = B // NCORES            # 4 batch rows per core
CHUNKS_PER_ROW = S // KSIZE            # 128
SEQS = ROWS_PER_CORE * CHUNKS_PER_ROW  # 512 chains per core
G = 2                                  # seq groups per core
NS = SEQS // G                         # 256 seqs per group
KT = 2                                 # contraction tiles (I/128 = H/128 = 2)

F32 = mybir.dt.float32
F16 = mybir.dt.float16
AF = mybir.ActivationFunctionType
OP = mybir.AluOpType

MM_DT = F16         # matmul operand + elementwise SBUF dtype
NP_MM_DT = np.float16


def build_nc():
    nc = bacc.Bacc("TRN2", target_bir_lowering=False, debug=False)

    # Inputs (host pre-transposed, contiguous per-DMA blocks).
    # xt[t, g, p, k, s] = x_shard[seq=g*NS+s, t, i=k*128+p]
    xt_d = nc.dram_tensor("xt", [KSIZE, G, 128, KT, NS], MM_DT, kind="ExternalInput")
    # wih_t[p, k, m] = W_ih[m, k*128+p]  (transposed weight, lhsT layout)
    wih_d = nc.dram_tensor("wih_t", [128, KT, 3 * H], MM_DT, kind="ExternalInput")
    whh_d = nc.dram_tensor("whh_t", [128, KT, 3 * H], MM_DT, kind="ExternalInput")
    # brz[p, mi] = (b_ih+b_hh)[mi*128+p] for mi in 0..3 (r0,r1,z0,z1)
    brz_d = nc.dram_tensor("brz", [128, 4], F32, kind="ExternalInput")
    # bhn[p, m] = b_hh[2H + m*128 + p]; bin[p, m] = b_ih[2H + m*128 + p]
    bhn_d = nc.dram_tensor("bhn", [128, 2], F32, kind="ExternalInput")
    bin_d = nc.dram_tensor("bin", [128, 2], F32, kind="ExternalInput")
    # out[t, g, p, m, s] = h_t[seq=g*NS+s, hdim=m*128+p]
    out_d = nc.dram_tensor("out", [KSIZE, G, 128, 2, NS], MM_DT, kind="ExternalOutput")

    with tile.TileContext(nc) as tc:
        with (
            tc.tile_pool(name="consts", bufs=1) as consts,
            tc.tile_pool(name="xp", bufs=8) as xp,
            tc.tile_pool(name="ps", bufs=2, space="PSUM") as ps,
            tc.tile_pool(name="work", bufs=4) as work,
            tc.tile_pool(name="hp", bufs=4) as hp,
        ):
            wih = consts.tile([128, KT, 3 * H], MM_DT)
            nc.sync.dma_start(wih[:], wih_d.ap())
            whh = consts.tile([128, KT, 3 * H], MM_DT)
            nc.sync.dma_start(whh[:], whh_d.ap())
            brz = consts.tile([128, 4], F32)
            nc.sync.dma_start(brz[:], brz_d.ap())
            bhn = consts.tile([128, 2], F32)
            nc.sync.dma_start(bhn[:], bhn_d.ap())
            bin_ = consts.tile([128, 2], F32)
            nc.sync.dma_start(bin_[:], bin_d.ap())

            h_state = [None] * G
            for t in range(KSIZE):
                for g in range(G):
                    xs = xp.tile([128, KT, NS], MM_DT, tag="x")
                    nc.sync.dma_start(xs[:], xt_d.ap()[t, g])
                    xr = xs[:]
                    hr = None if t == 0 else h_state[g][:]

                    # PSUM banks: [128, 2, NS] f32 = one 2KB bank each.
                    bank_r = ps.tile([128, 2, NS], F32, tag="r")
                    bank_z = ps.tile([128, 2, NS], F32, tag="z")
                    bank_in = ps.tile([128, 2, NS], F32, tag="in")
                    bank_hn = None if t == 0 else ps.tile([128, 2, NS], F32, tag="hn")

                    # Matmuls. W row tiles: r halves mi=0,1; z mi=2,3; n mi=4,5.
                    # Emission (= PE priority) order follows the dependency
                    # chain: r first (its sigmoid leads), then hn (feeds tmp),
                    # then z / in (consumed later).
                    def mm_accum(bank_t, mi, m, with_h):
                        col = slice(mi * 128, (mi + 1) * 128)
                        n_mm = 2 * KT if with_h else KT
                        i_mm = 0
                        for k in range(KT):
                            nc.tensor.matmul(
                                bank_t[:, m, :], wih[:, k, col], xr[:, k, :],
                                start=(i_mm == 0), stop=(i_mm == n_mm - 1),
                            )
                            i_mm += 1
                        if with_h:
                            for k in range(KT):
                                nc.tensor.matmul(
                                    bank_t[:, m, :], whh[:, k, col], hr[:, k, :],
                                    start=False, stop=(i_mm == n_mm - 1),
                                )
                                i_mm += 1

                    def mm_h_only(bank_t, mi, m):
                        col = slice(mi * 128, (mi + 1) * 128)
                        for k in range(KT):
                            nc.tensor.matmul(
                                bank_t[:, m, :], whh[:, k, col], hr[:, k, :],
                                start=(k == 0), stop=(k == KT - 1),
                            )

                    for m in range(2):
                        mm_accum(bank_r, m, m, t > 0)
                    if t > 0:
                        for m in range(2):
                            mm_h_only(bank_hn, 4 + m, m)
                    for m in range(2):
                        mm_accum(bank_z, 2 + m, m, t > 0)
                    for m in range(2):
                        mm_accum(bank_in, 4 + m, m, False)

                    # Elementwise.
                    # Separate r / z tiles so tmp's read of r never waits
                    # on the (later) z sigmoid writes.
                    r_t = work.tile([128, 2, NS], MM_DT, tag="rg")
                    z_t = work.tile([128, 2, NS], MM_DT, tag="zg")
                    for mi in range(2):  # r halves first: r leads the chain
                        nc.scalar.activation(
                            r_t[:, mi, :], bank_r[:, mi, :], AF.Sigmoid,
                            bias=brz[:, mi : mi + 1],
                        )
                    for mi in range(2):  # z halves after (consumed late)
                        nc.scalar.activation(
                            z_t[:, mi, :], bank_z[:, mi, :], AF.Sigmoid,
                            bias=brz[:, 2 + mi : 3 + mi],
                        )

                    tmp = work.tile([128, 2, NS], MM_DT, tag="tmp")
                    pren = work.tile([128, 2, NS], MM_DT, tag="pren")
                    for m in range(2):
                        if t == 0:
                            # h=0: h-side n contribution is just b_hh_n.
                            nc.vector.tensor_scalar_mul(
                                tmp[:, m, :], r_t[:, m, :], bhn[:, m : m + 1]
                            )
                        else:
                            # tmp = (psum_hn + b_hh_n) * r
                            nc.vector.scalar_tensor_tensor(
                                tmp[:, m, :], bank_hn[:, m, :], bhn[:, m : m + 1],
                                r_t[:, m, :], op0=OP.add, op1=OP.mult,
                            )
                        # pre_n = (psum_in + b_ih_n) + tmp
                        nc.vector.scalar_tensor_tensor(
                            pren[:, m, :], bank_in[:, m, :], bin_[:, m : m + 1],
                            tmp[:, m, :], op0=OP.add, op1=OP.add,
                        )

                    n_t = work.tile([128, 2, NS], MM_DT, tag="n")
                    nc.scalar.activation(n_t[:], pren[:], AF.Tanh)

                    hnew = hp.tile([128, 2, NS], MM_DT, tag="h")
                    e = work.tile([128, 2, NS], MM_DT, tag="e")
                    if t == 0:
                        # h1 = n - z*n
                        nc.vector.tensor_tensor(e[:], z_t[:], n_t[:], op=OP.mult)
                        nc.vector.tensor_tensor(hnew[:], n_t[:], e[:], op=OP.subtract)
                    else:
                        d = work.tile([128, 2, NS], MM_DT, tag="d")
                        # h = n + z*(h_prev - n)
                        nc.vector.tensor_tensor(
                            d[:], h_state[g][:], n_t[:], op=OP.subtract
                        )
                        nc.vector.tensor_tensor(e[:], z_t[:], d[:], op=OP.mult)
                        nc.vector.tensor_tensor(hnew[:], e[:], n_t[:], op=OP.add)

                    nc.sync.dma_start(out_d.ap()[t, g], hnew[:])
                    h_state[g] = hnew

    nc.compile()
    return nc


_NC_CACHE = None


def _get_nc():
    global _NC_CACHE
    if _NC_CACHE is None:
        _NC_CACHE = build_nc()
    return _NC_CACHE


def _prep_shared(W_ih, W_hh, b_ih, b_hh):
    wih_t = np.ascontiguousarray(
        W_ih.T.reshape(KT, 128, 3 * H).transpose(1, 0, 2)
    ).astype(NP_MM_DT)
    whh_t = np.ascontiguousarray(
        W_hh.T.reshape(KT, 128, 3 * H).transpose(1, 0, 2)
    ).astype(NP_MM_DT)
    bsum = b_ih + b_hh
    brz = np.ascontiguousarray(bsum[: 2 * H].reshape(4, 128).T)
    bhn = np.ascontiguousarray(b_hh[2 * H :].reshape(2, 128).T)
    bin_ = np.ascontiguousarray(b_ih[2 * H :].reshape(2, 128).T)
    return wih_t, whh_t, brz, bhn, bin_


def _prep_core_inputs(x, shared, core):
    wih_t, whh_t, brz, bhn, bin_ = shared
    xc = x[core * ROWS_PER_CORE : (core + 1) * ROWS_PER_CORE]  # [4, S, I]
    xc = xc.reshape(SEQS, KSIZE, I)
    # xt[t, g, p, k, s] = xc[g*NS+s, t, k*128+p]
    xt = np.ascontiguousarray(
        xc.reshape(G, NS, KSIZE, KT, 128).transpose(2, 0, 4, 3, 1)
    ).astype(NP_MM_DT)
    return {
        "xt": xt,
        "wih_t": wih_t,
        "whh_t": whh_t,
        "brz": brz,
        "bhn": bhn,
        "bin": bin_,
    }


def kernel(x, W_ih, W_hh, b_ih, b_hh, ksize):
    x = np.asarray(x, dtype=np.float32)
    W_ih = np.asarray(W_ih, dtype=np.float32)
    W_hh = np.asarray(W_hh, dtype=np.float32)
    b_ih = np.asarray(b_ih, dtype=np.float32)
    b_hh = np.asarray(b_hh, dtype=np.float32)
    assert int(ksize) == KSIZE and x.shape == (B, S, I)

    shared = _prep_shared(W_ih, W_hh, b_ih, b_hh)
    in_maps = [_prep_core_inputs(x, shared, c) for c in range(NCORES)]
    nc = _get_nc()
    res = run_bass_kernel_spmd(nc, in_maps, core_ids=list(range(NCORES)))

    out = np.empty((B, S, H), dtype=np.float32)
    for c in range(NCORES):
        oc = np.asarray(res.results[c]["out"]).astype(np.float32)  # [t,g,p,m,s]
        # h[seq=g*NS+s, t, hdim=m*128+p]
        hc = oc.transpose(1, 4, 0, 3, 2).reshape(SEQS, KSIZE, H)
        out[c * ROWS_PER_CORE : (c + 1) * ROWS_PER_CORE] = hc.reshape(
            ROWS_PER_CORE, S, H
        )
    return out



# revision 2
# speedup vs baseline: 1.0207x; 1.0207x over previous
"""Trainium2 Bass kernel for nn_LocalRNN (local GRU, chunked scan).

Problem: B=32, S=2048, I=H=256, ksize=16. Each ksize-chunk runs a GRU from
h0=0, so the 32*128=4096 chunks are independent length-16 GRU chains.

Sharding: data-parallel over chunks — core c gets batch rows [4c:4c+4],
i.e. 512 chains. Weights replicated.

Per-core kernel layout ("transposed"): gate/hidden dim on partitions, chain
(seq) index on the free dim. Per step t and seq-group g (2 groups x 256 seqs):

  gates[3H, seqs] = W_ih @ x_t^T + W_hh @ h_{t-1}^T     (PSUM accumulation)
  r = sigmoid(psum_r + (b_ih+b_hh)_r)                    (ScalarE, bias port)
  z = sigmoid(psum_z + (b_ih+b_hh)_z)
  n = tanh((psum_in + b_ih_n) + r*(psum_hn + b_hh_n))    (fused DVE stt ops)
  h = n + z*(h_prev - n)

Matmul emission order per group-step: ALL x-side matmuls first, then the
h-side block (r, hn, z). The x block needs no fresh dependencies, so the PE
has a deep queue of ready work covering the other group's elementwise chain
latency; the h-side r matmuls come first in the h block because sigmoid(r)
leads the elementwise chain.

h is written straight into a per-step staging tile [128, G, 2, NS] and
DMA'd out once per step (halves output DMA count); x tiles are DMA'd once
per step covering both groups; the weights land in one DMA triggered first
so the first matmul starts as early as possible.

Matmul operands and SBUF elementwise tensors are fp16 (DVE 2x mode; values
are O(1) so fp16 range is safe); PSUM accumulation is fp32. Host
pre-transposes x / weights into DMA-friendly contiguous blocks and inverts
the output layout at the end.
"""

import sys

for _p in ("/opt/trn_rl_repo", "/root/.axon_site"):
    if _p not in sys.path:
        sys.path.insert(0, _p)

import ml_dtypes
import numpy as np

import concourse.bass as bass  # noqa: F401
import concourse.tile as tile
from concourse import bacc, mybir
from concourse.bass_utils import run_bass_kernel_spmd

# Problem constants (hardcoded per harness contract).
B, S, I, H = 32, 2048, 256, 256
KSIZE = 16
NCORES = 8
ROWS_PER_CORE = B // NCORES            # 4 batch rows per core
CHUNKS_PER_ROW = S // KSIZE            # 128
SEQS = ROWS_PER_CORE * CHUNKS_PER_ROW  # 512 chains per core
G = 2                                  # seq groups per core
NS = SEQS // G                         # 256 seqs per group
KT = 2                                 # contraction tiles (I/128 = H/128 = 2)

F32 = mybir.dt.float32
F16 = mybir.dt.float16
AF = mybir.ActivationFunctionType
OP = mybir.AluOpType

MM_DT = F16         # matmul operand + elementwise SBUF dtype
NP_MM_DT = np.float16


def build_nc():
    nc = bacc.Bacc("TRN2", target_bir_lowering=False, debug=False)

    # Inputs (host pre-transposed, contiguous per-DMA blocks).
    # wt[p, k, w, m]: w=0 -> W_ih[m, k*128+p], w=1 -> W_hh[m, k*128+p]
    w_d = nc.dram_tensor("wt", [128, KT, 2, 3 * H], MM_DT, kind="ExternalInput")
    # bias8[p, j]: j=0..3 (b_ih+b_hh)[j*128+p] (r0,r1,z0,z1);
    #              j=4,5 b_hh[2H+m*128+p]; j=6,7 b_ih[2H+m*128+p]
    bias_d = nc.dram_tensor("bias8", [128, 8], F32, kind="ExternalInput")
    # xt[t, p, g, k, s] = x_shard[seq=g*NS+s, t, i=k*128+p]
    xt_d = nc.dram_tensor("xt", [KSIZE, 128, G, KT, NS], MM_DT, kind="ExternalInput")
    # out[t, p, g, m, s] = h_t[seq=g*NS+s, hdim=m*128+p]
    out_d = nc.dram_tensor("out", [KSIZE, 128, G, 2, NS], MM_DT, kind="ExternalOutput")

    with tile.TileContext(nc) as tc:
        with (
            tc.tile_pool(name="consts", bufs=1) as consts,
            tc.tile_pool(name="xp", bufs=4) as xp,
            tc.tile_pool(name="ps", bufs=2, space="PSUM") as ps,
            tc.tile_pool(name="work", bufs=4) as work,
            tc.tile_pool(name="stp", bufs=2) as stp,
        ):
            wt = consts.tile([128, KT, 2, 3 * H], MM_DT)
            nc.sync.dma_start(wt[:], w_d.ap())
            bias = consts.tile([128, 8], F32)
            nc.sync.dma_start(bias[:], bias_d.ap())

            h_state = [None] * G
            for t in range(KSIZE):
                xs = xp.tile([128, G, KT, NS], MM_DT, tag="x")
                nc.sync.dma_start(xs[:], xt_d.ap()[t])
                stage = stp.tile([128, G, 2, NS], MM_DT, tag="st")

                for g in range(G):
                    hr = None if t == 0 else h_state[g]

                    # PSUM banks: [128, 2, NS] f32 = one 2KB bank each.
                    bank_r = ps.tile([128, 2, NS], F32, tag="r")
                    bank_z = ps.tile([128, 2, NS], F32, tag="z")
                    bank_in = ps.tile([128, 2, NS], F32, tag="in")
                    bank_hn = None if t == 0 else ps.tile([128, 2, NS], F32, tag="hn")

                    # --- x block: all input-side matmuls (no fresh deps) ---
                    # W row tiles: r halves mi=0,1; z mi=2,3; n mi=4,5.
                    xparts = [
                        (bank_r, 0, 0), (bank_r, 1, 1),
                        (bank_z, 0, 2), (bank_z, 1, 3),
                        (bank_in, 0, 4), (bank_in, 1, 5),
                    ]
                    for bank, m, mi in xparts:
                        col = slice(mi * 128, (mi + 1) * 128)
                        xonly = t == 0 or mi >= 4  # group has no h-side part
                        for k in range(KT):
                            nc.tensor.matmul(
                                bank[:, m, :], wt[:, k, 0, col], xs[:, g, k, :],
                                start=(k == 0),
                                stop=(xonly and k == KT - 1),
                            )

                    # --- h block: recurrent matmuls (r first: its sigmoid
                    # leads the elementwise chain; then hn feeding tmp;
                    # z last, consumed late) ---
                    if t > 0:
                        hparts = [
                            (bank_r, 0, 0, False), (bank_r, 1, 1, False),
                            (bank_hn, 0, 4, True), (bank_hn, 1, 5, True),
                            (bank_z, 0, 2, False), (bank_z, 1, 3, False),
                        ]
                        for bank, m, mi, fresh in hparts:
                            col = slice(mi * 128, (mi + 1) * 128)
                            for k in range(KT):
                                nc.tensor.matmul(
                                    bank[:, m, :], wt[:, k, 1, col], hr[:, k, :],
                                    start=(fresh and k == 0),
                                    stop=(k == KT - 1),
                                )

                    # --- Elementwise ---
                    r_t = work.tile([128, 2, NS], MM_DT, tag="rg")
                    z_t = work.tile([128, 2, NS], MM_DT, tag="zg")
                    for mi in range(2):  # r halves first: r leads the chain
                        nc.scalar.activation(
                            r_t[:, mi, :], bank_r[:, mi, :], AF.Sigmoid,
                            bias=bias[:, mi : mi + 1],
                        )
                    for mi in range(2):  # z halves after (consumed late)
                        nc.scalar.activation(
                            z_t[:, mi, :], bank_z[:, mi, :], AF.Sigmoid,
                            bias=bias[:, 2 + mi : 3 + mi],
                        )

                    tmp = work.tile([128, 2, NS], MM_DT, tag="tmp")
                    pren = work.tile([128, 2, NS], MM_DT, tag="pren")
                    for m in range(2):
                        if t == 0:
                            # h=0: h-side n contribution is just b_hh_n.
                            nc.vector.tensor_scalar_mul(
                                tmp[:, m, :], r_t[:, m, :], bias[:, 4 + m : 5 + m]
                            )
                        else:
                            # tmp = (psum_hn + b_hh_n) * r
                            nc.vector.scalar_tensor_tensor(
                                tmp[:, m, :], bank_hn[:, m, :],
                                bias[:, 4 + m : 5 + m],
                                r_t[:, m, :], op0=OP.add, op1=OP.mult,
                            )
                        # pre_n = (psum_in + b_ih_n) + tmp
                        nc.vector.scalar_tensor_tensor(
                            pren[:, m, :], bank_in[:, m, :],
                            bias[:, 6 + m : 7 + m],
                            tmp[:, m, :], op0=OP.add, op1=OP.add,
                        )

                    n_t = work.tile([128, 2, NS], MM_DT, tag="n")
                    nc.scalar.activation(n_t[:], pren[:], AF.Tanh)

                    hnew = stage[:, g]
                    e = work.tile([128, 2, NS], MM_DT, tag="e")
                    if t == 0:
                        # h1 = n - z*n
                        nc.vector.tensor_tensor(e[:], z_t[:], n_t[:], op=OP.mult)
                        nc.vector.tensor_tensor(hnew, n_t[:], e[:], op=OP.subtract)
                    else:
                        d = work.tile([128, 2, NS], MM_DT, tag="d")
                        # h = n + z*(h_prev - n)
                        nc.vector.tensor_tensor(d[:], hr[:], n_t[:], op=OP.subtract)
                        nc.vector.tensor_tensor(e[:], z_t[:], d[:], op=OP.mult)
                        nc.vector.tensor_tensor(hnew, e[:], n_t[:], op=OP.add)

                    h_state[g] = hnew

                nc.sync.dma_start(out_d.ap()[t], stage[:])

    nc.compile()
    return nc


_NC_CACHE = None


def _get_nc():
    global _NC_CACHE
    if _NC_CACHE is None:
        _NC_CACHE = build_nc()
    return _NC_CACHE


def _prep_shared(W_ih, W_hh, b_ih, b_hh):
    # wt[p, k, w, m]
    wih_t = W_ih.T.reshape(KT, 128, 3 * H).transpose(1, 0, 2)  # [128, KT, 3H]
    whh_t = W_hh.T.reshape(KT, 128, 3 * H).transpose(1, 0, 2)
    wt = np.ascontiguousarray(
        np.stack([wih_t, whh_t], axis=2)
    ).astype(NP_MM_DT)  # [128, KT, 2, 3H]
    bsum = b_ih + b_hh
    bias8 = np.concatenate(
        [
            bsum[: 2 * H].reshape(4, 128).T,
            b_hh[2 * H :].reshape(2, 128).T,
            b_ih[2 * H :].reshape(2, 128).T,
        ],
        axis=1,
    )
    bias8 = np.ascontiguousarray(bias8).astype(np.float32)  # [128, 8]
    return wt, bias8


def _prep_core_inputs(x, shared, core):
    wt, bias8 = shared
    xc = x[core * ROWS_PER_CORE : (core + 1) * ROWS_PER_CORE]  # [4, S, I]
    xc = xc.reshape(SEQS, KSIZE, I)
    # xt[t, p, g, k, s] = xc[g*NS+s, t, k*128+p]
    xt = np.ascontiguousarray(
        xc.reshape(G, NS, KSIZE, KT, 128).transpose(2, 4, 0, 3, 1)
    ).astype(NP_MM_DT)
    return {"xt": xt, "wt": wt, "bias8": bias8}


def kernel(x, W_ih, W_hh, b_ih, b_hh, ksize):
    x = np.asarray(x, dtype=np.float32)
    W_ih = np.asarray(W_ih, dtype=np.float32)
    W_hh = np.asarray(W_hh, dtype=np.float32)
    b_ih = np.asarray(b_ih, dtype=np.float32)
    b_hh = np.asarray(b_hh, dtype=np.float32)
    assert int(ksize) == KSIZE and x.shape == (B, S, I)

    shared = _prep_shared(W_ih, W_hh, b_ih, b_hh)
    in_maps = [_prep_core_inputs(x, shared, c) for c in range(NCORES)]
    nc = _get_nc()
    res = run_bass_kernel_spmd(nc, in_maps, core_ids=list(range(NCORES)))

    out = np.empty((B, S, H), dtype=np.float32)
    for c in range(NCORES):
        oc = np.asarray(res.results[c]["out"]).astype(np.float32)  # [t,p,g,m,s]
        # h[seq=g*NS+s, t, hdim=m*128+p]
        hc = oc.transpose(2, 4, 0, 3, 1).reshape(SEQS, KSIZE, H)
        out[c * ROWS_PER_CORE : (c + 1) * ROWS_PER_CORE] = hc.reshape(
            ROWS_PER_CORE, S, H
        )
    return out
